# revision 2
# baseline (speedup 1.0000x reference)
"""Trainium2 Bass kernel for the CP-PINN tensor reconstruction problem.

Computes, for xs (3,320,1) and three per-axis MLP weight stacks:
    f_d = MLP_d(xs[d])            (320, 64)   [tanh MLP: 1->128->128->128->64]
    out[a,b,c] = sum_r f_0[a,r] * f_1[b,r] * f_2[c,r]   ->  (320, 320, 320) f32

Strategy: data-parallel over the output's first axis across 8 NeuronCores
(40 a-points per core, no collectives).  Rewrite of the 43.5 us pair-copy
baseline targeting the per-core HBM-write floor (8.2 MB fp16 / 360 GB/s
= 22.8 us):

  - MLP runs fully in float32r (xs is DMA'd into an f32r-declared dram
    tensor -- same bits, no conversion copy), so every 320-wide layer
    matmul is 1 cycle/row instead of 4.  Final-layer bias-adds moved to
    VectorE (tensor_scalar_add) so ScalarE only does the 9 tanhs.
  - Khatri-Rao products f0[r,j]*f1[r,b] run on the POOL/GPSIMD engine
    (SBUF-only op), freeing VectorE/ScalarE for PSUM drains.
  - CP reconstruction: 100 K=64 fp16 matmuls kr_chunk(64,128)^T @
    f2(64,320), four per 4-bank PSUM quad tile (2 bufs = all 8 banks).
    Each quad is drained by ONE fp16-downcasting quad-copy, alternating
    ScalarE/VectorE, into 8-chunk staging tiles (bufs=3).
  - Output streams as fp16 in 14 batched DMAs (8 chunks = 655 KB each,
    640 B HBM runs) all on the otherwise-idle SP HWDGE ring, cutting
    DMA-issue sequencer time ~4x vs 50 two-chunk DMAs.
  - The bench loop unrolls UNROLL bodies per tc.For_i iteration with
    double-buffered factor/staging tiles, so the ~5 us MLP head of body
    i+1 overlaps the DMA drain tail of body i, and the loop's all-engine
    reset barrier amortizes.

Output is written fp16 and upcast on host (CP values are O(1); fp16
rounding gives ~6e-4 rel_l2, ~30x inside the 2e-2 gate).
"""

import sys

if "/opt/trn_rl_repo" not in sys.path:
    sys.path.insert(0, "/opt/trn_rl_repo")

import numpy as np

import concourse.bacc as bacc
import concourse.mybir as mybir
from concourse import tile
from concourse.bass_utils import run_bass_kernel_spmd

DIMS = 3
N = 320          # points per coordinate axis
R = 64           # CP rank
H = 128          # hidden width
NCORES = 8
NA = N // NCORES          # a-points per core (40)
NROWS = NA * N            # output rows per core (12800)
MCH = 128                 # (a,b)-rows per matmul chunk
NCHUNK = NROWS // MCH     # 100
NHALF = NCHUNK // 2       # 50 chunks per row-half (lo = a<20, hi = a>=20)
PSB = 512                 # f32 elements per PSUM bank
GROUPS = (8, 8, 8, 8, 8, 8, 2)   # chunks per output DMA, per half

UNROLL = 2               # bodies per For_i iteration in the bench loop

# Packed-weights column layout (one (128, WCOLS) f32 tensor):
#   [0,384)    w1 (3 x 128 cols)        [384,768)  w2
#   [768,960)  w3 (3 x 64 cols)
#   [960,963) b0  [963,966) b1  [966,969) b2  [969,972) b3 (dup both halves)
#   [972,1356) w0 (row 0 only, 3 x 128 cols)
#   [1356,1740) w3 duplicated [w3|w3] (3 x 128 cols) — lets the d=1/2 final
#   layers write both partition halves with ONE full-col-group fp32r matmul.
W1_OFF, W2_OFF, W3_OFF = 0, 384, 768
B0_OFF, B1_OFF, B2_OFF, B3_OFF = 960, 963, 966, 969
W0_OFF, W3D_OFF, WCOLS = 972, 1356, 1740
# Packed-x layout: (1, 680) = x0(40) | x1(320) | x2(320)
X0_OFF, X1_OFF, X2_OFF, XCOLS = 0, NA, NA + N, NA + 2 * N

F32 = mybir.dt.float32
F32R = mybir.dt.float32r
F16 = mybir.dt.float16
TANH = mybir.ActivationFunctionType.Tanh

KR_ENGINE = "pool"       # "pool" (gpsimd) or "dve"

_PROG = None


def _build_program(loop=1, variant="full"):
    """loop>1 wraps the compute body in a Tile hardware For_i that repeats
    it `loop` times (UNROLL bodies per iteration) in one NEFF launch."""
    nc = bacc.Bacc("TRN2", target_bir_lowering=False)

    # xp is declared float32r (same 4-byte layout as the f32 host array) so
    # layer-0 matmuls run at f32r speed with no on-chip conversion.
    xp = nc.dram_tensor("xp", [1, XCOLS], F32R, kind="ExternalInput")
    wp = nc.dram_tensor("wp", [H, WCOLS], F32, kind="ExternalInput")
    out = nc.dram_tensor("out", [NROWS, N], F16, kind="ExternalOutput")

    with tile.TileContext(nc) as tc:
        with (
            tc.tile_pool(name="consts", bufs=1) as consts,
            tc.tile_pool(name="work", bufs=2) as work,
            tc.tile_pool(name="stage", bufs=3) as stagep,
            tc.tile_pool(name="cp_ps", bufs=2, space="PSUM") as cp_ps,
        ):
            wp_sb = consts.tile([H, WCOLS], F32)
            nc.sync.dma_start(wp_sb[:], wp[:, :])
            # fp32r copy of the weights for the PE (1 cycle/row vs 4 for
            # fp32 at N>=256); biases stay read from the f32 copy.
            wp_r = consts.tile([H, WCOLS], F32R)
            nc.vector.tensor_copy(wp_r[:], wp_sb[:])

            import contextlib
            if loop > 1 and loop % UNROLL == 0:
                bodies, trip = UNROLL, loop // UNROLL
            else:
                bodies, trip = 1, loop
            loop_cm = (tc.For_i(0, trip, 1,
                                hint_engines=(mybir.EngineType.PE,))
                       if loop > 1 else contextlib.nullcontext())
            with loop_cm:
                for _ in range(bodies):
                    _emit_body(nc, tc, consts, work, stagep, cp_ps,
                               xp, out, wp_sb, wp_r, variant)

    nc.compile()
    return nc


def _emit_body(nc, tc, consts, work, stagep, cp_ps,
               xp, out, wp_sb, wp_r, variant="full"):
    outv = out[:, :].rearrange("(m p) c -> p m c", p=MCH)

    def quad(name):
        # 4-bank (8 KB/partition) PSUM tile; bufs=2 -> all 8 banks
        return cp_ps.tile([MCH, 4 * PSB], F32, name=name, tag="cpq")

    if variant == "empty":
        z = work.tile([1, 1], F32, name="z", tag="z")
        nc.vector.memset(z[:], 0.0)
        return

    xp_sb = work.tile([1, XCOLS], F32R, name="xp_sb", tag="xp_sb")
    nc.sync.dma_start(xp_sb[:], xp[:, :])

    # Factor matrices in rank-major layout across both partition halves.
    # f1/f2: halves are duplicates.  f0: low half holds a in [0,20),
    # high half a in [20,40) -> KR ops engage all 128 partitions.
    f0_sb = work.tile([2 * R, NA // 2], F32, name="f0", tag="f0")
    f1_sb = work.tile([2 * R, N], F16, name="f1", tag="f1")
    f2_sb = work.tile([2 * R, N], F16, name="f2", tag="f2")

    # --- MLP: three dims interleaved layer-by-layer, all f32r on PE ---
    dims = [(0, X0_OFF, NA), (1, X1_OFF, N), (2, X2_OFF, N)]
    h_cur = {d: xp_sb[:, xoff:xoff + npts] for d, xoff, npts in dims}
    for li, (w_off, b_off, krows) in enumerate((
            (W0_OFF, B0_OFF, 1), (W1_OFF, B1_OFF, H), (W2_OFF, B2_OFF, H))):
        for d, _, npts in dims:
            ps = quad(f"ps{li}_{d}")
            nc.tensor.matmul(
                ps[:, 0:npts],
                wp_r[0:krows, w_off + d * H:w_off + (d + 1) * H],
                h_cur[d], start=True, stop=True)
            hdt, htag = ((F32, "h2_0") if (li == 2 and d == 0)
                         else (F32R, f"h_{d}"))
            h = work.tile([H, npts], hdt, name=f"h{li}_{d}", tag=htag)
            nc.scalar.activation(h[:], ps[:, 0:npts], TANH,
                                 bias=wp_sb[:, b_off + d:b_off + d + 1])
            h_cur[d] = h
    # Final layer.  d=1/2: one fp32r matmul with duplicated [w3|w3] weights
    # writes both partition halves at once.  d=0: the halves need DIFFERENT
    # a-ranges, so two plain-f32 matmuls (N=20, cheap) via col-group tiling.
    # Bias adds on VectorE (tensor_scalar_add) to keep ScalarE tanh-only.
    for d, _, npts in dims:
        ps = quad(f"psf_{d}")
        if d == 0:
            w3 = wp_sb[:, W3_OFF:W3_OFF + R]
            nc.tensor.matmul(ps[0:R, 0:NA // 2], w3, h_cur[0][:, 0:NA // 2],
                             start=True, stop=True, tile_position=(0, 0))
            nc.tensor.matmul(ps[R:2 * R, 0:NA // 2], w3,
                             h_cur[0][:, NA // 2:NA],
                             start=True, stop=True, tile_position=(0, R))
            nc.vector.tensor_scalar_add(
                f0_sb[:], ps[:, 0:NA // 2],
                wp_sb[:, B3_OFF:B3_OFF + 1])
        else:
            w3d = wp_r[:, W3D_OFF + d * H:W3D_OFF + (d + 1) * H]
            nc.tensor.matmul(ps[:, 0:N], w3d, h_cur[d],
                             start=True, stop=True)
            f_sb = f1_sb if d == 1 else f2_sb
            nc.vector.tensor_scalar_add(
                f_sb[:], ps[:, 0:N], wp_sb[:, B3_OFF + d:B3_OFF + d + 1])

    if variant == "mlp_only":
        return

    # Khatri-Rao: kr[p, j*N + b] = f1[p, b] * f0[p, j] on POOL (gpsimd),
    # emitted just-in-time ahead of the consuming matmuls.
    kr_sb = work.tile([2 * R, (NA // 2) * N], F16, name="kr", tag="kr")
    kr_eng = (nc.gpsimd.tensor_scalar_mul if KR_ENGINE == "pool"
              else nc.vector.tensor_scalar_mul)
    kr_state = [0]

    def emit_kr_upto(a_need):
        while kr_state[0] < min(a_need, NA // 2):
            j = kr_state[0]
            kr_eng(kr_sb[:, j * N:(j + 1) * N], f1_sb[:, :],
                   f0_sb[:, j:j + 1])
            kr_state[0] += 1

    if variant == "mlp_kr":
        emit_kr_upto(NA // 2)
        return

    # --- CP reconstruction + fp16 staging + batched output DMA ---
    copy_engines = (nc.scalar.copy, nc.vector.tensor_copy)
    copy_idx = [0]

    def drain(ps, stg, q0, nch):
        eng = copy_engines[copy_idx[0] % 2]
        copy_idx[0] += 1
        eng(stg[:, q0 * N:(q0 + nch) * N].rearrange("p (m c) -> p m c", c=N),
            ps[:, 0:nch * PSB].rearrange("p (m k) -> p m k", k=PSB)[:, :, 0:N])

    t0 = 0
    for s, gsz in enumerate(GROUPS):
        emit_kr_upto(-(-((t0 + gsz) * MCH) // N))
        stg_lo = stagep.tile([MCH, 8 * N], F16, name="stg_lo", tag="stg_lo")
        stg_hi = stagep.tile([MCH, 8 * N], F16, name="stg_hi", tag="stg_hi")
        nquads = gsz // 4 if gsz >= 4 else 1
        qw = 4 if gsz >= 4 else gsz
        for q in range(nquads):
            for half, stg in ((0, stg_lo), (1, stg_hi)):
                ps = quad(f"cp{s}_{q}_{half}")
                for k in range(qw):
                    c0 = (t0 + q * 4 + k) * MCH
                    nc.tensor.matmul(
                        ps[:, k * PSB:k * PSB + N],
                        kr_sb[half * R:(half + 1) * R, c0:c0 + MCH],
                        f2_sb[half * R:(half + 1) * R, :],
                        start=True, stop=True)
                if variant != "no_copy":
                    drain(ps, stg, q * 4, qw)
        if variant in ("no_copy", "no_dma"):
            t0 += gsz
            continue
        nc.sync.dma_start(
            outv[:, t0:t0 + gsz, :],
            stg_lo[:, 0:gsz * N].rearrange("p (m c) -> p m c", c=N))
        nc.sync.dma_start(
            outv[:, NHALF + t0:NHALF + t0 + gsz, :],
            stg_hi[:, 0:gsz * N].rearrange("p (m c) -> p m c", c=N))
        t0 += gsz


def _get_program():
    global _PROG
    if _PROG is None:
        _PROG = _build_program()
    return _PROG


def _pack_weights(W0, b0, W1, b1, W2, b2, W3, b3):
    wp = np.zeros((H, WCOLS), np.float32)
    for d in range(DIMS):
        wp[:, W1_OFF + d * H:W1_OFF + (d + 1) * H] = W1[d]
        wp[:, W2_OFF + d * H:W2_OFF + (d + 1) * H] = W2[d]
        wp[:, W3_OFF + d * R:W3_OFF + (d + 1) * R] = W3[d]
        wp[:, B0_OFF + d] = b0[d]
        wp[:, B1_OFF + d] = b1[d]
        wp[:, B2_OFF + d] = b2[d]
        wp[0:R, B3_OFF + d] = b3[d]
        wp[R:2 * R, B3_OFF + d] = b3[d]
        wp[0, W0_OFF + d * H:W0_OFF + (d + 1) * H] = W0[d, 0]
        wp[:, W3D_OFF + d * H:W3D_OFF + d * H + R] = W3[d]
        wp[:, W3D_OFF + d * H + R:W3D_OFF + (d + 1) * H] = W3[d]
    return wp


def _make_in_maps(xs, W0, b0, W1, b1, W2, b2, W3, b3):
    f = lambda x: np.ascontiguousarray(np.asarray(x), dtype=np.float32)
    xs = f(xs)
    wp = _pack_weights(f(W0), f(b0), f(W1), f(b1), f(W2), f(b2), f(W3), f(b3))
    in_maps = []
    for i in range(NCORES):
        x = np.empty((1, XCOLS), np.float32)
        x[0, X0_OFF:X0_OFF + NA] = xs[0, i * NA:(i + 1) * NA, 0]
        x[0, X1_OFF:X1_OFF + N] = xs[1, :, 0]
        x[0, X2_OFF:X2_OFF + N] = xs[2, :, 0]
        in_maps.append({"xp": x, "wp": wp})
    return in_maps


def run_spmd(inputs_kwargs, **run_kwargs):
    """Build (cached) program, run on all 8 cores; returns BassKernelResults."""
    nc = _get_program()
    in_maps = _make_in_maps(**inputs_kwargs)
    return run_bass_kernel_spmd(nc, in_maps, core_ids=list(range(NCORES)),
                                **run_kwargs)


def kernel(xs, W0, b0, W1, b1, W2, b2, W3, b3):
    res = run_spmd(dict(xs=xs, W0=W0, b0=b0, W1=W1, b1=b1,
                        W2=W2, b2=b2, W3=W3, b3=b3))
    slabs = [r["out"].astype(np.float32).reshape(NA, N, N)
             for r in res.results]
    return np.concatenate(slabs, axis=0)


# revision 35
# speedup vs baseline: 3.8158x; 3.8158x over previous
"""Trainium2 Bass kernel for the CP-PINN tensor reconstruction problem.

Computes, for xs (3,320,1) and three per-axis MLP weight stacks:
    f_d = MLP_d(xs[d])            (320, 64)   [tanh MLP: 1->128->128->128->64]
    out[a,b,c] = sum_r f_0[a,r] * f_1[b,r] * f_2[c,r]   ->  (320, 320, 320) f32

Strategy: data-parallel over the output's first axis across 8 NeuronCores
(40 a-points per core, no collectives).  Measured per-core resource
floors (probe variants, this box): output-DMA stream ~25.4 us for the
8.2 MB fp16 slab (~323 GB/s real, independent of descriptor run length
640 B..32 KB and of ring/group choices), ScalarE pair-drain 706 ns,
VectorE 870 ns (dual-engine drain floor ~22 us).  The kernel holds
~29-30 us/body against those:

  - MLP fully in float32r (xs is DMA'd into an f32r-declared dram tensor
    -- same bits, no conversion), 1 cycle/row on the PE at N>=256;
    tanhs on ScalarE, final bias-adds on ScalarE (scalar.add).
  - CP reconstruction: 100 K=64 fp16 matmuls kr_chunk(64,128)^T @
    f2(64,320).  Chunk pairs land CONTIGUOUSLY in PSUM (first chunk at
    bank offset 192 = PSB-N, second at the next bank's offset 0) so each
    drain is a single 2D 640-element fp16-downcasting copy from a shared
    4-deep 2-bank PSUM ring.  Drains STRICTLY alternate lo->ScalarE /
    hi->VectorE: every irregular split tried (28:22 Bresenham, 3:2
    period-5, per-engine rings) measurably stalls the ring.
  - Khatri-Rao products on VectorE (tensor_scalar_mul, 16-bit perf
    mode).  GPSIMD measured ~4.4 us/op on HW -- unusable.
  - Output streams as fp16 in 14 batched DMAs (8 chunks = 655 KB) on the
    otherwise-idle SP HWDGE ring; staging tiles bufs=4.
  - Bench loop: UNROLL=8 serial bodies per For_i iteration (barrier
    amortized ~0.8 us/body).  Cross-body tile double-buffering lets the
    next body's xp DMA + PE matmuls ride the previous DMA tail.

Measured dead ends (HW, paired benches): instruction-level head
injection into the stream (+1..4 us), MLP-before-stream reordering
(+5), KR split to ScalarE (+2), UNROLL=16 (+5), PE p-state keep-alive
dummy matmuls (+26 -- the cost model's 3 us ramp-to-2.4 GHz does not
materialize under a drain-paced stream), row-block DMA layouts (~0),
lo/hi-interleaved matmul emission (~0).

Output is written fp16 and upcast on host (CP values are O(1); fp16
rounding gives ~6.7e-4 rel_l2, ~30x inside the 2e-2 gate).
"""

import sys

if "/opt/trn_rl_repo" not in sys.path:
    sys.path.insert(0, "/opt/trn_rl_repo")

import numpy as np

import concourse.bacc as bacc
import concourse.mybir as mybir
from concourse import tile
from concourse.bass_utils import run_bass_kernel_spmd

DIMS = 3
N = 320          # points per coordinate axis
R = 64           # CP rank
H = 128          # hidden width
NCORES = 8
NA = N // NCORES          # a-points per core (40)
NROWS = NA * N            # output rows per core (12800)
MCH = 128                 # (a,b)-rows per matmul chunk
NCHUNK = NROWS // MCH     # 100
NHALF = NCHUNK // 2       # 50 chunks per row-half (lo = a<20, hi = a>=20)
PSB = 512                 # f32 elements per PSUM bank
POFF = PSB - N            # in-bank offset making chunk pairs contiguous
GROUPS = (8, 8, 8, 8, 8, 8, 2)   # chunks per output DMA, per half

UNROLL = 8               # bodies per For_i iteration in the bench loop

# Packed-weights column layout (one (128, WCOLS) f32 tensor):
#   [0,384)    w1 (3 x 128 cols)        [384,768)  w2
#   [768,960)  w3 (3 x 64 cols)
#   [960,963) b0  [963,966) b1  [966,969) b2  [969,972) b3 (dup both halves)
#   [972,1356) w0 (row 0 only, 3 x 128 cols)
#   [1356,1740) w3 duplicated [w3|w3] (3 x 128 cols) — lets the d=1/2 final
#   layers write both partition halves with ONE full-col-group fp32r matmul.
W1_OFF, W2_OFF, W3_OFF = 0, 384, 768
B0_OFF, B1_OFF, B2_OFF, B3_OFF = 960, 963, 966, 969
W0_OFF, W3D_OFF, WCOLS = 972, 1356, 1740
# Packed-x layout: (1, 680) = x0(40) | x1(320) | x2(320)
X0_OFF, X1_OFF, X2_OFF, XCOLS = 0, NA, NA + N, NA + 2 * N

F32 = mybir.dt.float32
F32R = mybir.dt.float32r
F16 = mybir.dt.float16
TANH = mybir.ActivationFunctionType.Tanh

KR_ENGINE = "dve"        # "pool" (gpsimd) is ~4.4us/op on HW — unusable

_PROG = None


class _Ctx:
    """Shared emission context for one program build."""

    def __init__(self, nc, consts, work, stagep, cp_ps, xp, out, wp_sb, wp_r):
        self.nc = nc
        self.consts, self.work = consts, work
        self.stagep, self.cp_ps = stagep, cp_ps
        self.xp, self.wp_sb, self.wp_r = xp, wp_sb, wp_r
        self.out = out
        self.outv = out[:, :].rearrange("(m p) c -> p m c", p=MCH)

    ring_bufs = 4
    per_engine_rings = False

    def pair(self, name, half=None):
        # 2-bank PSUM pair tile; bufs=4 -> all 8 banks, ring depth 4
        # (shared by stream pairs and head matmuls, as measured fastest).
        # per_engine_rings: lo(ACT) and hi(DVE) pairs in separate 2x2-bank
        # rings so each drain engine's chain releases independently.
        if self.per_engine_rings:
            if half is None:   # head matmuls alternate between the rings
                self._alt = 1 - getattr(self, "_alt", 0)
                half = self._alt
            return self.cp_ps.tile([MCH, 2 * PSB], F32, name=name,
                                   tag=f"pp{half}", bufs=2)
        return self.cp_ps.tile([MCH, 2 * PSB], F32, name=name, tag="pp",
                               bufs=self.ring_bufs)

    def head_ps(self, name):
        # 1-bank PSUM tile for MLP/final matmuls (outputs <= 320 cols),
        # separate from the stream ring so an injected head matmul never
        # holds a stream slot while waiting for its tanh.
        return self.cp_ps.tile([MCH, PSB], F32, name=name, tag="hp",
                               bufs=2)


def _head_ops(cx, tiles):
    """Build the next body's head as a list of (engine_tag, closure)
    instruction emitters in dependency-chain order: xp DMA, 12 MLP
    matmuls + 9 tanhs, 3 final bias-adds, 20 Khatri-Rao products.
    Tags let the stream inject each op at a slot where its engine reaches
    it just after its dependency resolved (in-order engines must never
    stall on an injected op)."""
    nc = cx.nc
    wp_sb, wp_r = cx.wp_sb, cx.wp_r
    st = {}
    ops = []

    def op(tag, fn):
        ops.append((tag, fn))

    def emit_xp():
        xp_sb = cx.work.tile([1, XCOLS], F32R, name="xp_sb", tag="xp_sb")
        nc.sync.dma_start(xp_sb[:], cx.xp[:, :])
        st["x"] = xp_sb

    op("sp", emit_xp)

    dims = [(0, X0_OFF, NA), (1, X1_OFF, N), (2, X2_OFF, N)]

    def emit_mm(li, d, w_off, krows, npts, xoff):
        def fn():
            ps = cx.pair(f"ps{li}_{d}")
            h_in = (st["x"][:, xoff:xoff + npts] if li == 0
                    else st[f"h{li - 1}_{d}"][:])
            nc.tensor.matmul(
                ps[:, 0:npts],
                wp_r[0:krows, w_off + d * H:w_off + (d + 1) * H],
                h_in, start=True, stop=True)
            st[f"ps{li}_{d}"] = ps
        return fn

    def emit_tanh(li, d, b_off, npts):
        def fn():
            hdt, htag = ((F32, "h2_0") if (li == 2 and d == 0)
                         else (F32R, f"h_{d}"))
            h = cx.work.tile([H, npts], hdt, name=f"h{li}_{d}", tag=htag)
            nc.scalar.activation(h[:], st[f"ps{li}_{d}"][:, 0:npts], TANH,
                                 bias=wp_sb[:, b_off + d:b_off + d + 1])
            st[f"h{li}_{d}"] = h
        return fn

    for li, (w_off, b_off, krows) in enumerate((
            (W0_OFF, B0_OFF, 1), (W1_OFF, B1_OFF, H), (W2_OFF, B2_OFF, H))):
        for d, xoff, npts in dims:
            op("pe", emit_mm(li, d, w_off, krows, npts, xoff))
            op("act", emit_tanh(li, d, b_off, npts))

    # Final layer.  d=1/2: one fp32r matmul with duplicated [w3|w3] weights
    # writes both partition halves at once.  d=0: two plain-f32 matmuls
    # (N=20, cheap) via col-group tiling.  Bias adds on VectorE.
    def emit_fin_mm(d):
        def fn():
            ps = cx.pair(f"psf_{d}")
            if d == 0:
                w3 = wp_sb[:, W3_OFF:W3_OFF + R]
                h = st["h2_0"]
                nc.tensor.matmul(ps[0:R, 0:NA // 2], w3, h[:, 0:NA // 2],
                                 start=True, stop=True, tile_position=(0, 0))
                nc.tensor.matmul(ps[R:2 * R, 0:NA // 2], w3,
                                 h[:, NA // 2:NA],
                                 start=True, stop=True, tile_position=(0, R))
            else:
                w3d = wp_r[:, W3D_OFF + d * H:W3D_OFF + (d + 1) * H]
                nc.tensor.matmul(ps[:, 0:N], w3d, st[f"h2_{d}"][:],
                                 start=True, stop=True)
            st[f"psf_{d}"] = ps
        return fn

    def emit_fin_add(d):
        def fn():
            npts = NA // 2 if d == 0 else N
            nc.scalar.add(tiles[f"f{d}"][:], st[f"psf_{d}"][:, 0:npts],
                          wp_sb[:, B3_OFF + d:B3_OFF + d + 1])
        return fn

    for d in (0, 1, 2):
        op("pe", emit_fin_mm(d))
        op("act", emit_fin_add(d))

    def emit_kr(j):
        def fn():
            if KR_ENGINE == "split" and j % 3 == 2:
                nc.scalar.activation(
                    tiles["kr"][:, j * N:(j + 1) * N], tiles["f1"][:, :],
                    mybir.ActivationFunctionType.Copy,
                    scale=tiles["f0"][:, j:j + 1])
            else:
                nc.vector.tensor_scalar_mul(
                    tiles["kr"][:, j * N:(j + 1) * N], tiles["f1"][:, :],
                    tiles["f0"][:, j:j + 1])
        return fn

    for j in range(NA // 2):
        op("dve" if not (KR_ENGINE == "split" and j % 3 == 2) else "act",
           emit_kr(j))

    return ops


class _Feeder:
    """Pulls head ops in chain order, but only emits an op when the
    current injection slot matches its engine tag (so in-order engines
    reach each injected op after its dependency has resolved)."""

    def __init__(self, ops):
        self.ops = list(ops)
        self.i = 0

    def take(self, tags, limit=1):
        n = 0
        while self.i < len(self.ops) and n < limit:
            tag, fn = self.ops[self.i]
            if tag not in tags:
                return n
            fn()
            self.i += 1
            n += 1
        return n

    def drain(self):
        while self.i < len(self.ops):
            self.ops[self.i][1]()
            self.i += 1


def _make_tiles(cx):
    """Factor tiles for one body (double-buffered via the work pool).
    f1/f2: rank-major, both partition halves duplicates.  f0: low half
    holds a in [0,20), high half a in [20,40)."""
    return {
        "f0": cx.work.tile([2 * R, NA // 2], F32, name="f0", tag="f0"),
        "f1": cx.work.tile([2 * R, N], F16, name="f1", tag="f1"),
        "f2": cx.work.tile([2 * R, N], F16, name="f2", tag="f2"),
        "kr": cx.work.tile([2 * R, (NA // 2) * N], F16, name="kr", tag="kr"),
    }


def _emit_probe(cx, variant):
    """Microbenchmark bodies: probe_cpA/B/AB time 50 pair drains on
    ACT / DVE / both; probe_dma times the 14-DMA output stream alone."""
    nc = cx.nc
    if variant.startswith("probe_cp"):
        which = variant[8:]
        stg = cx.stagep.tile([MCH, 8 * N], F16, name="pstg", tag="stg_lo")
        for i in range(NHALF):
            ps = cx.pair(f"pp{i}")
            nc.vector.memset(ps[:, POFF:POFF + 1], 0.0)
            eng = (nc.scalar.copy if which == "A"
                   else nc.vector.tensor_copy if which == "B"
                   else (nc.scalar.copy, nc.vector.tensor_copy)[i % 2])
            eng(stg[:, (i % 4) * 2 * N:(i % 4 + 1) * 2 * N],
                ps[:, POFF:POFF + 2 * N])
        return
    if variant.startswith("probe_rb"):
        # Row-block dst layout: partition p owns 50 consecutive HBM rows
        # per half -> per-DMA descriptor runs of gsz*640 B (vs 640 B).
        # probe_rb1: one whole-half DMA each (32 KB/partition runs).
        groups = (50,) if variant == "probe_rb1" else GROUPS
        out_lo = cx.out[0:NHALF * MCH, :].rearrange(
            "(p m) c -> p m c", p=MCH)
        out_hi = cx.out[NHALF * MCH:NROWS, :].rearrange(
            "(p m) c -> p m c", p=MCH)
        gmax = max(groups)
        t0 = 0
        for s, gsz in enumerate(groups):
            stg_lo = cx.stagep.tile([MCH, gmax * N], F16, name="stg_lo",
                                    tag="stg_lo", bufs=2)
            stg_hi = cx.stagep.tile([MCH, gmax * N], F16, name="stg_hi",
                                    tag="stg_hi", bufs=2)
            nc.vector.memset(stg_lo[:, 0:1], 1.0)
            nc.vector.memset(stg_hi[:, 0:1], 1.0)
            nc.sync.dma_start(
                out_lo[:, t0:t0 + gsz, :],
                stg_lo[:, 0:gsz * N].rearrange("p (m c) -> p m c", c=N))
            nc.sync.dma_start(
                out_hi[:, t0:t0 + gsz, :],
                stg_hi[:, 0:gsz * N].rearrange("p (m c) -> p m c", c=N))
            t0 += gsz
        return
    if variant.startswith("probe_dma"):
        # probe_dma[_16][_pool]: output-DMA stream alone; _16 = 16-chunk
        # groups, _pool = hi-half DMAs issued on the Pool SWDGE ring.
        groups = ((16, 16, 16, 2) if "_16" in variant
                  else (4,) * 12 + (2,) if "_4" in variant else GROUPS)
        ring2 = nc.gpsimd if "_pool" in variant else nc.sync
        gmax = max(groups)
        t0 = 0
        for s, gsz in enumerate(groups):
            stg_lo = cx.stagep.tile([MCH, gmax * N], F16, name="stg_lo",
                                    tag="stg_lo", bufs=2)
            stg_hi = cx.stagep.tile([MCH, gmax * N], F16, name="stg_hi",
                                    tag="stg_hi", bufs=2)
            nc.vector.memset(stg_lo[:, 0:1], 1.0)
            nc.vector.memset(stg_hi[:, 0:1], 1.0)
            nc.sync.dma_start(
                cx.outv[:, t0:t0 + gsz, :],
                stg_lo[:, 0:gsz * N].rearrange("p (m c) -> p m c", c=N))
            ring2.dma_start(
                cx.outv[:, NHALF + t0:NHALF + t0 + gsz, :],
                stg_hi[:, 0:gsz * N].rearrange("p (m c) -> p m c", c=N))
            t0 += gsz
        return


def _emit_stream(cx, tiles, feeder, variant="full"):
    """CP reconstruction stream for one body: 50 lo + 50 hi chunks as
    contiguous PSUM pairs drained into 8-chunk staging tiles, DMA'd out
    on the SP ring.  Drains are split ACT:DVE = 28:22 (measured 706 vs
    870 ns/pair), balancing against ACT's 9 tanhs and DVE's KR+finals.
    Head ops of the NEXT body are injected at per-engine slots: PE ops
    before a round's matmuls, ACT ops after an ACT drain, DVE ops after
    a DVE drain -- each lands ~1 round after its dependency emitted."""
    nc = cx.nc
    f2_sb, kr_sb = tiles["f2"], tiles["kr"]
    if feeder is not None:
        feeder.take(("sp",))   # prime the next body's xp DMA immediately
    rnd = [0]

    def slot(tags, limit=1):
        if feeder is not None and rnd[0] >= 3:
            feeder.take(tags, limit)

    # Strict ACT/DVE alternation: any irregular doubles (tested 28:22
    # Bresenham, 3:2 period-5) stall the shared PSUM ring badly on HW.
    drain_eng = {i: (nc.scalar.copy if i % 2 == 0 else
                     nc.vector.tensor_copy) for i in range(NHALF)}

    def dummy_mm():
        # PE p-state keep-alive: an independent 512-col matmul into a
        # scratch bank so the tensor clock never ramps down while the
        # stream is drain-paced (model: >3us continuous busy = 2.4 GHz).
        ps_d = cx.head_ps("pdum")
        nc.tensor.matmul(ps_d[:, 0:PSB], cx.wp_r[0:R, 0:MCH],
                         cx.wp_r[0:R, 0:PSB], start=True, stop=True)

    t0 = 0
    pi = 0
    for s, gsz in enumerate(GROUPS):
        stg_lo = cx.stagep.tile([MCH, 8 * N], F16, name="stg_lo",
                                tag="stg_lo", bufs=4)
        stg_hi = cx.stagep.tile([MCH, 8 * N], F16, name="stg_hi",
                                tag="stg_hi", bufs=4)
        for q in range(gsz // 2):
            rnd[0] += 1
            slot(("pe", "sp"), 2)
            if variant == "ilv":
                # emit the round's 4 matmuls alternating lo/hi so
                # consecutive PE instructions use row groups 0/64 and
                # pipeline concurrently in the array
                ps2 = {0: cx.pair(f"cp{s}_{q}_0", 0),
                       1: cx.pair(f"cp{s}_{q}_1", 1)}
                for k in range(2):
                    for half in (0, 1):
                        c0 = (t0 + q * 2 + k) * MCH
                        nc.tensor.matmul(
                            ps2[half][:, POFF + k * N:POFF + (k + 1) * N],
                            kr_sb[half * R:(half + 1) * R, c0:c0 + MCH],
                            f2_sb[half * R:(half + 1) * R, :],
                            start=True, stop=True)
                for half, stg in ((0, stg_lo), (1, stg_hi)):
                    eng = drain_eng[pi]
                    pi += 1
                    eng(stg[:, q * 2 * N:(q * 2 + 2) * N],
                        ps2[half][:, POFF:POFF + 2 * N])
                    slot(("act",) if eng is nc.scalar.copy else ("dve",))
                continue
            if variant == "dummy":
                dummy_mm()
            for half, stg in ((0, stg_lo), (1, stg_hi)):
                ps = cx.pair(f"cp{s}_{q}_{half}", half)
                ncol = N // 2 if variant == "halfmm" else N
                for k in range(2):
                    c0 = (t0 + q * 2 + k) * MCH
                    nc.tensor.matmul(
                        ps[:, POFF + k * N:POFF + k * N + ncol],
                        kr_sb[half * R:(half + 1) * R, c0:c0 + MCH],
                        f2_sb[half * R:(half + 1) * R, 0:ncol],
                        start=True, stop=True)
                if variant == "dummy":
                    dummy_mm()
                if variant != "no_copy":
                    eng = drain_eng[pi]
                    pi += 1
                    dcol = N if variant == "halfdrain" else 2 * N
                    eng(stg[:, q * 2 * N:q * 2 * N + dcol],
                        ps[:, POFF:POFF + dcol])
                    slot(("act",) if eng is nc.scalar.copy else ("dve",))
        if variant in ("no_copy", "no_dma"):
            t0 += gsz
            continue
        nc.sync.dma_start(
            cx.outv[:, t0:t0 + gsz, :],
            stg_lo[:, 0:gsz * N].rearrange("p (m c) -> p m c", c=N))
        nc.sync.dma_start(
            cx.outv[:, NHALF + t0:NHALF + t0 + gsz, :],
            stg_hi[:, 0:gsz * N].rearrange("p (m c) -> p m c", c=N))
        t0 += gsz
    if feeder is not None:
        feeder.drain()


def _emit_head_eager(cx, variant="full"):
    """Emit a full head with no interleaving; returns its factor tiles."""
    tiles = _make_tiles(cx)
    ops = _head_ops(cx, tiles)
    n_kr = NA // 2
    if variant == "mlp_only":
        ops = ops[:-n_kr]
    for _, fn in ops:
        fn()
    return tiles


def _build_program(loop=1, variant="full"):
    """loop>1 wraps the compute in a Tile hardware For_i; each iteration
    holds UNROLL software-pipelined bodies (stream of body i overlapped
    with head of body i+1)."""
    nc = bacc.Bacc("TRN2", target_bir_lowering=False)

    # xp is declared float32r (same 4-byte layout as the f32 host array) so
    # layer-0 matmuls run at f32r speed with no on-chip conversion.
    xp = nc.dram_tensor("xp", [1, XCOLS], F32R, kind="ExternalInput")
    wp = nc.dram_tensor("wp", [H, WCOLS], F32, kind="ExternalInput")
    out = nc.dram_tensor("out", [NROWS, N], F16, kind="ExternalOutput")

    with tile.TileContext(nc) as tc:
        with (
            tc.tile_pool(name="consts", bufs=1) as consts,
            tc.tile_pool(name="work", bufs=2) as work,
            tc.tile_pool(name="stage", bufs=3) as stagep,
            tc.tile_pool(name="cp_ps", bufs=2, space="PSUM") as cp_ps,
        ):
            wp_sb = consts.tile([H, WCOLS], F32)
            nc.sync.dma_start(wp_sb[:], wp[:, :])
            # fp32r copy of the weights for the PE (1 cycle/row vs 4 for
            # fp32 at N>=256); biases stay read from the f32 copy.
            wp_r = consts.tile([H, WCOLS], F32R)
            nc.vector.tensor_copy(wp_r[:], wp_sb[:])

            cx = _Ctx(nc, consts, work, stagep, cp_ps, xp, out, wp_sb, wp_r)

            if loop > 1 and loop % UNROLL == 0 and UNROLL % 2 == 0:
                bodies, trip = UNROLL, loop // UNROLL
            else:
                bodies, trip = 1, loop

            if variant.startswith("probe"):
                import contextlib
                cm = (tc.For_i(0, trip, 1,
                               hint_engines=(mybir.EngineType.PE,))
                      if loop > 1 else contextlib.nullcontext())
                with cm:
                    for _ in range(bodies):
                        _emit_probe(cx, variant)
            elif trip > 1 or bodies > 1:
                if variant == "mlp_only":
                    with tc.For_i(0, trip, 1,
                                  hint_engines=(mybir.EngineType.PE,)):
                        for _ in range(bodies):
                            _emit_head_eager(cx, variant)
                elif variant == "serial":
                    with tc.For_i(0, trip, 1,
                                  hint_engines=(mybir.EngineType.PE,)):
                        for _ in range(bodies):
                            tiles = _emit_head_eager(cx)
                            _emit_stream(cx, tiles, None, "full")
                elif variant == "serial2":
                    # software pipeline without instruction injection:
                    # MLP of body X+1 before stream X (tanh chain rides
                    # the stream's start), its KR after (DVE post-drain)
                    tiles = _emit_head_eager(cx)
                    n_kr = NA // 2
                    with tc.For_i(0, trip, 1,
                                  hint_engines=(mybir.EngineType.PE,)):
                        for _ in range(bodies):
                            nxt = _make_tiles(cx)
                            ops = _head_ops(cx, nxt)
                            for _, fn in ops[:-n_kr]:
                                fn()
                            _emit_stream(cx, tiles, None, "full")
                            for _, fn in ops[-n_kr:]:
                                fn()
                            tiles = nxt
                elif variant == "inject":
                    tiles = _emit_head_eager(cx)
                    with tc.For_i(0, trip, 1,
                                  hint_engines=(mybir.EngineType.PE,)):
                        for _ in range(bodies):
                            nxt = _make_tiles(cx)
                            feeder = _Feeder(_head_ops(cx, nxt))
                            _emit_stream(cx, tiles, feeder, "full")
                            tiles = nxt
                else:
                    # default: serial emission per body (fastest on HW)
                    if variant == "dummy":
                        cx.ring_bufs = 3
                    if variant == "perring":
                        cx.per_engine_rings = True
                    with tc.For_i(0, trip, 1,
                                  hint_engines=(mybir.EngineType.PE,)):
                        for _ in range(bodies):
                            tiles = _emit_head_eager(cx)
                            _emit_stream(cx, tiles, None, variant)
            else:
                tiles = _emit_head_eager(cx, variant)
                if variant != "mlp_only":
                    _emit_stream(cx, tiles, None, variant)

    nc.compile()
    return nc


def _drain(feeders):
    while feeders:
        try:
            next(feeders[0])
        except StopIteration:
            feeders.pop(0)


def _get_program():
    global _PROG
    if _PROG is None:
        _PROG = _build_program()
    return _PROG


def _pack_weights(W0, b0, W1, b1, W2, b2, W3, b3):
    wp = np.zeros((H, WCOLS), np.float32)
    for d in range(DIMS):
        wp[:, W1_OFF + d * H:W1_OFF + (d + 1) * H] = W1[d]
        wp[:, W2_OFF + d * H:W2_OFF + (d + 1) * H] = W2[d]
        wp[:, W3_OFF + d * R:W3_OFF + (d + 1) * R] = W3[d]
        wp[:, B0_OFF + d] = b0[d]
        wp[:, B1_OFF + d] = b1[d]
        wp[:, B2_OFF + d] = b2[d]
        wp[0:R, B3_OFF + d] = b3[d]
        wp[R:2 * R, B3_OFF + d] = b3[d]
        wp[0, W0_OFF + d * H:W0_OFF + (d + 1) * H] = W0[d, 0]
        wp[:, W3D_OFF + d * H:W3D_OFF + d * H + R] = W3[d]
        wp[:, W3D_OFF + d * H + R:W3D_OFF + (d + 1) * H] = W3[d]
    return wp


def _make_in_maps(xs, W0, b0, W1, b1, W2, b2, W3, b3):
    f = lambda x: np.ascontiguousarray(np.asarray(x), dtype=np.float32)
    xs = f(xs)
    wp = _pack_weights(f(W0), f(b0), f(W1), f(b1), f(W2), f(b2), f(W3), f(b3))
    in_maps = []
    for i in range(NCORES):
        x = np.empty((1, XCOLS), np.float32)
        x[0, X0_OFF:X0_OFF + NA] = xs[0, i * NA:(i + 1) * NA, 0]
        x[0, X1_OFF:X1_OFF + N] = xs[1, :, 0]
        x[0, X2_OFF:X2_OFF + N] = xs[2, :, 0]
        in_maps.append({"xp": x, "wp": wp})
    return in_maps


def run_spmd(inputs_kwargs, **run_kwargs):
    """Build (cached) program, run on all 8 cores; returns BassKernelResults."""
    nc = _get_program()
    in_maps = _make_in_maps(**inputs_kwargs)
    return run_bass_kernel_spmd(nc, in_maps, core_ids=list(range(NCORES)),
                                **run_kwargs)


def kernel(xs, W0, b0, W1, b1, W2, b2, W3, b3):
    res = run_spmd(dict(xs=xs, W0=W0, b0=b0, W1=W1, b1=b1,
                        W2=W2, b2=b2, W3=W3, b3=b3))
    slabs = [r["out"].astype(np.float32).reshape(NA, N, N)
             for r in res.results]
    return np.concatenate(slabs, axis=0)
